# revision 62
# baseline (speedup 1.0000x reference)
"""ARIMAX(3,1,2)+exog recurrence over (B=1024, 1, T=8192), f32.

Strategy: pure data-parallel over batch (128 rows/core x 8 cores). The
order-4 linear recurrence along T is factored into chunked linear
algebra: per 128-step chunk a causal-convolution matmul (impulse
response of the AR polynomial precomputed on host in f64) gives the
particular solution; cross-chunk coupling is a 4-dim state propagated
for all 64 chunks at once with one small matmul against a precomputed
power-matrix; a final injection matmul adds the state response. A
cancelling +/- scaled injection pair reproduces the f32 overflow->NaN
semantics of the sequential reference scan.

Kernel I/O is [B,1,T] float32; output [B,1,T] float32.
"""

import sys
import numpy as np

for _p in ("/opt/trn_rl_repo",):
    if _p not in sys.path:
        sys.path.insert(0, _p)

import concourse.bass as bass
import concourse.mybir as mybir
from concourse import bacc, tile
from concourse.bass_utils import run_bass_kernel_spmd

# ---- static problem geometry (hardcoded per problem spec) ----
B_TOT, T_LEN = 1024, 8192
NCORES = 8
BC = B_TOT // NCORES          # 128 batch rows per core
L = 128                       # chunk length along T
C = T_LEN // L                # 64 chunks
NG = 4                        # chunks per group (one PSUM bank of 512)
G = C // NG                   # 16 groups
HALF = C // 2                 # 32 chunks per D-half
AUX_EXPO = 1.5                # aux overflow-kill scale exponent

F32 = mybir.dt.float32
F32R = mybir.dt.float32r

# matmul dtype knobs (bitcast to float32r runs PE 4x faster at N>=256)
MM_CONV_F32R = False
MM_INJ_F32R = False
MM_S_F32R = False
TRANS_F32R = False


def _host_precompute(ar, ma, xcoef, drift):
    """f64 host precompute of all coefficient matrices (O(L^2 + C^2))."""
    # lag coeffs exactly as the reference computes them (f32 convolve)
    dcoef = np.array([-1.0, 1.0], np.float32)
    c32 = np.convolve(ar.astype(np.float32), dcoef)[:4].astype(np.float32)
    c = c32.astype(np.float64)

    # impulse response h[0..L]
    h = np.zeros(L + 1, np.float64)
    h[0] = 1.0
    for k in range(1, L + 1):
        acc = 0.0
        for j in range(4):
            lag = 4 - j
            if k - lag >= 0:
                acc += c[j] * h[k - lag]
        h[k] = acc

    # state responses g_j[tau] (state = last 4 outputs before chunk)
    Gm = np.zeros((L, 4), np.float64)
    for j in range(4):
        r = np.zeros(L + 4, np.float64)
        r[j] = 1.0
        for tau in range(L):
            t = tau + 4
            r[t] = c[0] * r[t - 4] + c[1] * r[t - 3] + c[2] * r[t - 2] + c[3] * r[t - 1]
        Gm[:, j] = r[4:]
    Psi = Gm[L - 4:, :]  # s_{c+1}[i] = d_c[i] + sum_j Psi[i,j] s_c[j]

    with np.errstate(over="ignore", invalid="ignore"):
        powers = [np.eye(4)]
        for _ in range(C - 1):
            powers.append(Psi @ powers[-1])

        # Mstate in permuted layouts:
        # row (k) layout per half: p = 32*j + cc  -> chunk c' = 32*half + cc, state j
        # col (m) layout per half: p = 32*i + CC  -> chunk Cfull = 32*half + CC, slot i
        MstA = np.zeros((128, 256), np.float64)  # rows: k-half0; cols: [Sp0 | Sp1]
        MstB = np.zeros((128, 128), np.float64)  # rows: k-half1; cols: Sp1
        for colh in range(2):
            for i in range(4):
                for CCn in range(32):
                    Cfull = 32 * colh + CCn
                    m = 128 * colh + 32 * i + CCn
                    # contributions from k-half0 (chunks 0..31)
                    for j in range(4):
                        for cc in range(32):
                            cp = cc
                            if cp < Cfull:
                                MstA[32 * j + cc, m] = powers[Cfull - 1 - cp][i, j]
                    if colh == 1:
                        for j in range(4):
                            for cc in range(32):
                                cp = 32 + cc
                                if cp < Cfull:
                                    MstB[32 * j + cc, m - 128] = powers[Cfull - 1 - cp][i, j]

        ma64 = ma.astype(np.float64)
        hm = np.zeros(L + 1, np.float64)
        hm[:L] += ma64[1] * h[:L]
        hm[1:] += ma64[0] * h[:L]
        idx = np.arange(L)
        lagm = idx[None, :] - idx[:, None]  # tau - k
        Hm = np.where(lagm >= 0, hm[np.clip(lagm, 0, L)], 0.0)
        Hx = np.where(lagm >= 0, float(xcoef[0]) * h[np.clip(lagm, 0, L - 1)], 0.0)
        hb = (ma64[0] * h[:L]).reshape(1, L)           # weight of y[cL-1] at tau
        rd = (float(drift[0]) * np.cumsum(h[:L])).reshape(L, 1)

        # boundary-term state feed: d_c excludes the boundary contribution
        # (it is injected at phase 2), so states get it via W @ U where
        # U[c', b] = y[b, 128 c' + 127] and w_{C,c'} = Psi^{C-1-c'} @ hbv.
        hbv = hb[0, L - 4:]                            # [4]
        wvecs = {}
        for p in range(C):
            wvecs[p] = powers[p] @ hbv                 # [4] (i-indexed)
        WA = np.zeros((HALF, 256), np.float64)         # rows c' in half0
        WB = np.zeros((HALF, 128), np.float64)         # rows c' in half1
        # boundary of chunk c' uses u_{c'-1}; only c' >= 1 has one, so the
        # weight of u_p in s_C is Psi^{C-2-p} @ hbv for p <= C-2.
        for colh in range(2):
            for i in range(4):
                for CCn in range(HALF):
                    Cfull = HALF * colh + CCn
                    col = 128 * colh + HALF * i + CCn
                    for cc in range(HALF):
                        if cc <= Cfull - 2:            # u_p with p = cc (half0)
                            WA[cc, col] = wvecs[Cfull - 2 - cc][i]
                        if colh == 1 and HALF + cc <= Cfull - 2:
                            WB[cc, col - 128] = wvecs[Cfull - 2 - (HALF + cc)][i]

        rho = float(np.max(np.abs(np.roots([1.0, -c[3], -c[2], -c[1], -c[0]]))))
        Gaux = Gm / max(rho, 1.0) ** AUX_EXPO

        # main inject lhsT [5, 128]: rows 0-3 g_j (state response), row 4 hb
        # (boundary y[cL-1] response). The +/- aux pair accumulates into the
        # same PSUM as two separate f32r matmuls: exact cancellation when
        # finite, inf + (-inf) = NaN in PSUM on overflow (PE in-array K-chain
        # does NOT form NaN, so the pair must be PSUM-level).
        Gmain = np.zeros((128, L), np.float64)
        Gmain[0:4, :] = Gm.T
        Gmain[4, :] = hb[0]
        GauxP = np.zeros((128, L), np.float64)
        GauxP[0:4, :] = Gaux.T
        GauxN = -GauxP

        f = lambda x: np.ascontiguousarray(np.asarray(x, np.float32))
        return dict(
            Hm=f(Hm), Hx=f(Hx), rd=f(rd), Gmain=f(Gmain),
            GauxP=f(GauxP), GauxN=f(GauxN),
            MstA=f(MstA), MstB=f(MstB), WA=f(WA), WB=f(WB),
        )


def _r(ap, want):
    """optionally bitcast an AP to float32r"""
    return ap.bitcast(F32R) if want else ap


def _build_program():
    nc = bacc.Bacc("TRN2", target_bir_lowering=False, debug=False,
                   num_devices=NCORES)

    # y/ex arrive chunk-transposed from the host: row k, column 128*c + b
    # holds y[b, 128*c + k] (so conv matmuls read them directly as rhs)
    y_d = nc.dram_tensor("y", [BC, T_LEN], F32, kind="ExternalInput")
    ex_d = nc.dram_tensor("ex", [BC, T_LEN], F32, kind="ExternalInput")
    Hm_d = nc.dram_tensor("Hm", [L, L], F32, kind="ExternalInput")
    Hx_d = nc.dram_tensor("Hx", [L, L], F32, kind="ExternalInput")
    rd_d = nc.dram_tensor("rd", [L, 1], F32, kind="ExternalInput")
    Gm_d = nc.dram_tensor("Gmain", [128, L], F32, kind="ExternalInput")
    GxP_d = nc.dram_tensor("GauxP", [128, L], F32R, kind="ExternalInput")
    GxN_d = nc.dram_tensor("GauxN", [128, L], F32R, kind="ExternalInput")
    MstA_d = nc.dram_tensor("MstA", [128, 256], F32, kind="ExternalInput")
    MstB_d = nc.dram_tensor("MstB", [128, 128], F32, kind="ExternalInput")
    WA_d = nc.dram_tensor("WA", [HALF, 256], F32, kind="ExternalInput")
    WB_d = nc.dram_tensor("WB", [HALF, 128], F32, kind="ExternalInput")
    zed_d = nc.dram_tensor("zed", [1, 128], F32, kind="ExternalInput")
    out_d = nc.dram_tensor("out_t", [T_LEN, BC], F32, kind="ExternalOutput")

    HB = HALF * 128  # 4096 columns per half

    with tile.TileContext(nc) as tc:
        with (
            tc.tile_pool(name="consts", bufs=1) as consts,
            tc.tile_pool(name="persist", bufs=1) as persist,

            tc.tile_pool(name="outring", bufs=3) as outring,
            tc.tile_pool(name="small", bufs=1) as small,
            tc.tile_pool(name="pcv", bufs=3, space="PSUM") as pcv,
            tc.tile_pool(name="pin", bufs=3, space="PSUM") as pin,
            tc.tile_pool(name="psp", bufs=1, space="PSUM") as psp,
            tc.tile_pool(name="dram", bufs=1, space="DRAM") as dpool,
        ):
            Hm_t = consts.tile([L, L], F32, tag="Hm")
            Hx_t = consts.tile([L, L], F32, tag="Hx")
            rd_t = consts.tile([L, 1], F32, tag="rd")
            Gm_t = consts.tile([128, L], F32, tag="Gm")
            GxP_t = consts.tile([128, L], F32R, tag="GxP")
            GxN_t = consts.tile([128, L], F32R, tag="GxN")
            MstA_t = consts.tile([128, 256], F32, tag="MstA")
            MstB_t = consts.tile([128, 128], F32, tag="MstB")
            WA_t = consts.tile([HALF, 256], F32, tag="WA")
            WB_t = consts.tile([HALF, 128], F32, tag="WB")
            def load_early_consts():
                for t_, d_ in ((Hm_t, Hm_d), (Hx_t, Hx_d), (rd_t, rd_d)):
                    nc.sync.dma_start(t_[:], d_[:])

            def load_late_consts():
                # the half-0 state matmuls need these two first; the scalar
                # ring delivers them without queueing behind the bulk inputs
                nc.scalar.dma_start(MstA_t[:], MstA_d[:])
                nc.scalar.dma_start(WA_t[:], WA_d[:])
                for t_, d_ in ((Gm_t, Gm_d), (GxP_t, GxP_d), (GxN_t, GxN_d),
                               (MstB_t, MstB_d), (WB_t, WB_d)):
                    nc.scalar.dma_start(t_[:], d_[:])

            yT = persist.tile([128, T_LEN], F32, tag="yT")
            exT = persist.tile([128, T_LEN], F32, tag="exT")
            pT = persist.tile([128, T_LEN], F32, tag="pT")

            D0 = small.tile([128, 128], F32, tag="D0")
            D1 = small.tile([128, 128], F32, tag="D1")
            Sp0 = small.tile([128, 128], F32, tag="Sp0")
            Sp1 = small.tile([128, 128], F32, tag="Sp1")
            U0 = small.tile([HALF, 128], F32, tag="U0")
            U1 = small.tile([HALF, 128], F32, tag="U1")
            # inject rhs: S2m rows 0-3 states + row 4 boundary-u (f32);
            # S2x rows 0-3 states again, typed f32r for the aux pair
            S2m = small.tile([128, T_LEN], F32, tag="S2m")
            S2x = small.tile([128, T_LEN], F32R, tag="S2x")
            # zero the padding rows once (K padded to 128 so inject matmuls
            # stream at full rate; zero weights x zero rhs contribute nothing)
            nc.gpsimd.memset(S2m[:], 0.0)
            nc.gpsimd.memset(S2x[:].bitcast(F32), 0.0)

            scrD_0 = dpool.tile([1, 4 * HB], F32, tag="scrD0")
            scrD_1 = dpool.tile([1, 4 * HB], F32, tag="scrD1")
            scrS_0 = dpool.tile([1, 4 * HB], F32, tag="scrS0")
            scrS_1 = dpool.tile([1, 4 * HB], F32, tag="scrS1")
            scrD = [scrD_0, scrD_1]
            scrS = [scrS_0, scrS_1]

            def load_inputs():
                # ramped slice sizes: fast start, then efficient bulk
                bounds = [0, 512, 1536, 3072, 4608, 6400, 8192]
                for a, b in zip(bounds[:-1], bounds[1:]):
                    nc.sync.dma_start(yT[:, a:b], y_d[:, a:b])
                    nc.sync.dma_start(exT[:, a:b], ex_d[:, a:b])

            def phase1(g):
                sl = slice(g * 512, (g + 1) * 512)
                pp = pcv.tile([128, 512], F32, tag="pcv")
                nc.tensor.matmul(pp[:], Hm_t[:], yT[:, sl],
                                 start=True, stop=False)
                nc.tensor.matmul(pp[:], Hx_t[:], exT[:, sl],
                                 start=False, stop=True)
                if g % 2 == 0:
                    nc.vector.tensor_scalar_add(pT[:, sl], pp[:], rd_t[:, 0:1])
                else:
                    nc.scalar.activation(pT[:, sl], pp[:],
                                         mybir.ActivationFunctionType.Identity,
                                         bias=rd_t[:, 0:1], scale=1.0)

            def load_late_consts():
                # the half-0 state matmuls need these two first; the scalar
                # ring delivers them without queueing behind the bulk inputs
                nc.scalar.dma_start(MstA_t[:], MstA_d[:])
                nc.scalar.dma_start(WA_t[:], WA_d[:])
                for t_, d_ in ((Gm_t, Gm_d), (GxP_t, GxP_d), (GxN_t, GxN_d),
                               (MstB_t, MstB_d), (WB_t, WB_d)):
                    nc.scalar.dma_start(t_[:], d_[:])

            yT = persist.tile([128, T_LEN], F32, tag="yT")
            exT = persist.tile([128, T_LEN], F32, tag="exT")
            pT = persist.tile([128, T_LEN], F32, tag="pT")

            D0 = small.tile([128, 128], F32, tag="D0")
            D1 = small.tile([128, 128], F32, tag="D1")
            Sp0 = small.tile([128, 128], F32, tag="Sp0")
            Sp1 = small.tile([128, 128], F32, tag="Sp1")
            U0 = small.tile([HALF, 128], F32, tag="U0")
            U1 = small.tile([HALF, 128], F32, tag="U1")
            # inject rhs: S2m rows 0-3 states + row 4 boundary-u (f32);
            # S2x rows 0-3 states again, typed f32r for the aux pair
            S2m = small.tile([128, T_LEN], F32, tag="S2m")
            S2x = small.tile([128, T_LEN], F32R, tag="S2x")
            # zero the padding rows once (K padded to 128 so inject matmuls
            # stream at full rate; zero weights x zero rhs contribute nothing)
            nc.gpsimd.memset(S2m[:], 0.0)
            nc.gpsimd.memset(S2x[:].bitcast(F32), 0.0)

            scrD_0 = dpool.tile([1, 4 * HB], F32, tag="scrD0")
            scrD_1 = dpool.tile([1, 4 * HB], F32, tag="scrD1")
            scrS_0 = dpool.tile([1, 4 * HB], F32, tag="scrS0")
            scrS_1 = dpool.tile([1, 4 * HB], F32, tag="scrS1")
            scrD = [scrD_0, scrD_1]
            scrS = [scrS_0, scrS_1]

            def load_inputs():
                # ramped slice sizes: fast start, then efficient bulk
                bounds = [0, 512, 1536, 3072, 4608, 6400, 8192]
                for a, b in zip(bounds[:-1], bounds[1:]):
                    nc.sync.dma_start(yT[:, a:b], y_d[:, a:b])
                    nc.sync.dma_start(exT[:, a:b], ex_d[:, a:b])

            def phase1(g):
                sl = slice(g * 512, (g + 1) * 512)
                pp = pcv.tile([128, 512], F32, tag="pcv")
                nc.tensor.matmul(pp[:], Hm_t[:], yT[:, sl],
                                 start=True, stop=False)
                nc.tensor.matmul(pp[:], Hx_t[:], exT[:, sl],
                                 start=False, stop=True)
                if g % 2 == 0:
                    nc.vector.tensor_scalar_add(pT[:, sl], pp[:], rd_t[:, 0:1])
                else:
                    nc.scalar.activation(pT[:, sl], pp[:],
                                         mybir.ActivationFunctionType.Identity,
                                         bias=rd_t[:, 0:1], scale=1.0)

            def preload_state_rhs():
                # U rows and the boundary rhs row come straight from input DRAM
                nc.sync.dma_start(
                    U0[:], y_d[127:128, 0:HB]
                    .rearrange("one (p b) -> (one p) b", b=128))
                nc.sync.dma_start(
                    U1[:], y_d[127:128, HB:2 * HB]
                    .rearrange("one (p b) -> (one p) b", b=128))
                nc.sync.dma_start(S2m[4:5, 0:128], zed_d[:])
                nc.sync.dma_start(S2m[4:5, 128:2 * HB],
                                    y_d[127:128, 0:2 * HB - 128])

            def hs_gather(h):
                dma_eng = nc.scalar.dma_start
                cols = slice(h * HB, (h + 1) * HB)
                # single batched d-gather: [4 part, 4096] -> flat j-major
                dma_eng(
                    scrD[h][:].rearrange("one (j n) -> (one j) n", n=HB),
                    pT[124:128, cols])
                D = D0 if h == 0 else D1
                dma_eng(
                    D[:], scrD[h][:].rearrange("one (p b) -> (one p) b", b=128))

            def hs_compute(h):
                dma_eng = nc.scalar.dma_start
                cols = slice(h * HB, (h + 1) * HB)
                ps = psp.tile([128, 128], F32, tag="psp", name=f"ps{h}")
                if h == 0:
                    nc.tensor.matmul(ps[:], MstA_t[:, 0:128], D0[:],
                                     start=True, stop=False)
                    nc.tensor.matmul(ps[:], WA_t[:, 0:128], U0[:],
                                     start=False, stop=True)
                else:
                    nc.tensor.matmul(ps[:], MstA_t[:, 128:256], D0[:],
                                     start=True, stop=False)
                    nc.tensor.matmul(ps[:], MstB_t[:], D1[:],
                                     start=False, stop=False)
                    nc.tensor.matmul(ps[:], WA_t[:, 128:256], U0[:],
                                     start=False, stop=False)
                    nc.tensor.matmul(ps[:], WB_t[:], U1[:],
                                     start=False, stop=True)
                Sp = Sp0 if h == 0 else Sp1
                nc.vector.tensor_copy(Sp[:], ps[:])
                # single batched shuffle: Sp [128,128] -> flat i-major -> rows
                dma_eng(
                    scrS[h][:].rearrange("one (i n) -> (one i) n", n=HB),
                    Sp[:])
                dma_eng(
                    S2m[0:4, cols],
                    scrS[h][:].rearrange("one (i n) -> (one i) n", n=HB))
                dma_eng(
                    S2x[0:4, cols],
                    scrS[h][:].rearrange("one (i n) -> (one i) n",
                                         n=HB).bitcast(F32R))

            og_hold = [None]

            def phase2(g):
                sl = slice(g * 512, (g + 1) * 512)
                pj = pin.tile([128, 512], F32, tag="pin")
                nc.tensor.matmul(pj[:], Gm_t[:], S2m[:, sl],
                                 start=True, stop=False)
                nc.tensor.matmul(pj[:], GxP_t[:], S2x[:, sl],
                                 start=False, stop=False)
                nc.tensor.matmul(pj[:], GxN_t[:], S2x[:, sl],
                                 start=False, stop=True)
                if g % 2 == 0:
                    og_hold[0] = outring.tile([128, 1024], F32, tag="og",
                                              name=f"og{g}")
                og = og_hold[0]
                half = (g % 2) * 512
                nc.vector.tensor_add(og[:, half:half + 512], pT[:, sl], pj[:])
                if g % 2 == 1:
                    osl = slice((g - 1) * 512, (g + 1) * 512)
                    nc.sync.dma_start(
                        out_d[osl, :].rearrange("(c t) b -> t c b", c=2 * NG),
                        og[:].rearrange("t (c b) -> t c b", c=2 * NG))

            junk = small.tile([1, 4], F32, tag="junk")

            def warmup_dummies(n, tag):
                pdum = psp.tile([128, 512], F32, tag="pdum", name=f"pd{tag}")
                for i in range(n):
                    nc.tensor.matmul(pdum[:], Hm_t[:], yT[:, 0:512],
                                     start=(i == 0), stop=(i == n - 1))
                nc.vector.tensor_copy(junk[:], pdum[0:1, 0:4])
                nc.gpsimd.dma_start(scrD[0][0:1, 0:4], junk[:])

            load_early_consts()
            load_inputs()
            preload_state_rhs()
            load_late_consts()
            for g in range(G):
                phase1(g)
                if g == 7:
                    hs_gather(0)
                if g == 12:
                    hs_compute(0)
            hs_gather(1)
            for g in range(G // 2):
                phase2(g)
                if g == 3:
                    hs_compute(1)
            for g in range(G // 2, G):
                phase2(g)

    nc.finalize()
    return nc


_CACHE = {}
TRACE = False        # set True (e.g. from test.py) to capture neuron-profile
LAST_RESULT = None   # BassKernelResults of the last run


def kernel(y, exog, ar, ma, xcoef, drift):
    y = np.asarray(y)
    exog = np.asarray(exog)
    pc = _host_precompute(np.asarray(ar), np.asarray(ma),
                          np.asarray(xcoef), np.asarray(drift))

    if "nc" not in _CACHE:
        _CACHE["nc"] = _build_program()
    nc = _CACHE["nc"]

    # chunk-transpose on host: per core [k, 128*c + b] = y[b, 128*c + k]
    y4 = (y.reshape(NCORES, BC, C, L).transpose(0, 3, 2, 1)
          .reshape(NCORES, L, T_LEN).astype(np.float32))
    ex4 = (exog.reshape(NCORES, BC, C, L).transpose(0, 3, 2, 1)
           .reshape(NCORES, L, T_LEN).astype(np.float32))

    in_maps = []
    for i in range(NCORES):
        m = {
            "y": np.ascontiguousarray(y4[i]),
            "ex": np.ascontiguousarray(ex4[i]),
            "zed": np.zeros((1, 128), np.float32),
        }
        m.update(pc)  # const names match dram tensor names
        in_maps.append(m)

    res = run_bass_kernel_spmd(nc, in_maps, core_ids=list(range(NCORES)),
                               trace=TRACE)
    global LAST_RESULT
    LAST_RESULT = res

    out = np.empty((B_TOT, 1, T_LEN), np.float32)
    for i in range(NCORES):
        out[i * BC:(i + 1) * BC, 0, :] = res.results[i]["out_t"].T
    return out


if __name__ == "__main__":
    d = np.load("/root/problem/ref_inp.npz")
    inp = {k: d[k] for k in d.files}
    o = kernel(**inp)
    print("kernel ran, out shape", o.shape)
    ref = np.load("/root/problem/ref_out_np.npy")
    r = ref[:, 0, :]
    oo = o[:, 0, :]
    bf = np.isfinite(r) & np.isfinite(oo)
    rel = np.abs(oo - r)[bf] / (np.abs(r)[bf] + 1e-30)
    print("finite rel: max %.3g mean %.3g" % (rel.max(), rel.mean()))
    print("class mism:", (np.isfinite(r) != np.isfinite(oo)).sum(),
          "nan mism:", (np.isnan(r) != np.isnan(oo)).sum())
    np.save("/root/problem/last_out.npy", o)
    nm = (np.isnan(r) != np.isnan(oo))
    ts = np.where(nm.any(axis=0))[0]
    if len(ts):
        print("nan-mismatch t range", ts.min(), ts.max(), "ncols", len(ts))
    # finite-region error per chunk (diagnose which chunks are wrong)
    for c in [0, 1, 2, 3, 10, 32, 63]:
        s = slice(c * 128, (c + 1) * 128)
        b = np.isfinite(r[:, s]) & np.isfinite(oo[:, s])
        if b.any():
            e = np.abs(oo[:, s] - r[:, s])[b] / (np.abs(r[:, s])[b] + 1e-30)
            print(f"chunk {c}: n_finite {b.sum()} relmax {e.max():.3g} "
                  f"relmean {e.mean():.3g}")


# revision 66
# speedup vs baseline: 1.0770x; 1.0770x over previous
"""ARIMAX(3,1,2)+exog recurrence over (B=1024, 1, T=8192), f32.

Strategy: pure data-parallel over batch (128 rows/core x 8 cores). The
order-4 linear recurrence along T is factored into chunked linear
algebra: per 128-step chunk a causal-convolution matmul (impulse
response of the AR polynomial precomputed on host in f64) gives the
particular solution; cross-chunk coupling is a 4-dim state propagated
for all 64 chunks at once with one small matmul against a precomputed
power-matrix; a final injection matmul adds the state response. A
cancelling +/- scaled injection pair reproduces the f32 overflow->NaN
semantics of the sequential reference scan.

Kernel I/O is [B,1,T] float32; output [B,1,T] float32.
"""

import sys
import numpy as np

for _p in ("/opt/trn_rl_repo",):
    if _p not in sys.path:
        sys.path.insert(0, _p)

import concourse.bass as bass
import concourse.mybir as mybir
from concourse import bacc, tile
from concourse.bass_utils import run_bass_kernel_spmd

# ---- static problem geometry (hardcoded per problem spec) ----
B_TOT, T_LEN = 1024, 8192
NCORES = 8
BC = B_TOT // NCORES          # 128 batch rows per core
L = 128                       # chunk length along T
C = T_LEN // L                # 64 chunks
NG = 4                        # chunks per group (one PSUM bank of 512)
G = C // NG                   # 16 groups
HALF = C // 2                 # 32 chunks per D-half
AUX_EXPO = 1.5                # aux overflow-kill scale exponent

F32 = mybir.dt.float32
F32R = mybir.dt.float32r

# matmul dtype knobs (bitcast to float32r runs PE 4x faster at N>=256)
MM_CONV_F32R = False
MM_INJ_F32R = False
MM_S_F32R = False
TRANS_F32R = False


def _host_precompute(ar, ma, xcoef, drift):
    """f64 host precompute of all coefficient matrices (O(L^2 + C^2))."""
    # lag coeffs exactly as the reference computes them (f32 convolve)
    dcoef = np.array([-1.0, 1.0], np.float32)
    c32 = np.convolve(ar.astype(np.float32), dcoef)[:4].astype(np.float32)
    c = c32.astype(np.float64)

    # impulse response h[0..L]
    h = np.zeros(L + 1, np.float64)
    h[0] = 1.0
    for k in range(1, L + 1):
        acc = 0.0
        for j in range(4):
            lag = 4 - j
            if k - lag >= 0:
                acc += c[j] * h[k - lag]
        h[k] = acc

    # state responses g_j[tau] (state = last 4 outputs before chunk)
    Gm = np.zeros((L, 4), np.float64)
    for j in range(4):
        r = np.zeros(L + 4, np.float64)
        r[j] = 1.0
        for tau in range(L):
            t = tau + 4
            r[t] = c[0] * r[t - 4] + c[1] * r[t - 3] + c[2] * r[t - 2] + c[3] * r[t - 1]
        Gm[:, j] = r[4:]
    Psi = Gm[L - 4:, :]  # s_{c+1}[i] = d_c[i] + sum_j Psi[i,j] s_c[j]

    with np.errstate(over="ignore", invalid="ignore"):
        powers = [np.eye(4)]
        for _ in range(C - 1):
            powers.append(Psi @ powers[-1])

        # Mstate in permuted layouts:
        # row (k) layout per half: p = 32*j + cc  -> chunk c' = 32*half + cc, state j
        # col (m) layout per half: p = 32*i + CC  -> chunk Cfull = 32*half + CC, slot i
        MstA = np.zeros((128, 256), np.float64)  # rows: k-half0; cols: [Sp0 | Sp1]
        MstB = np.zeros((128, 128), np.float64)  # rows: k-half1; cols: Sp1
        for colh in range(2):
            for i in range(4):
                for CCn in range(32):
                    Cfull = 32 * colh + CCn
                    m = 128 * colh + 32 * i + CCn
                    # contributions from k-half0 (chunks 0..31)
                    for j in range(4):
                        for cc in range(32):
                            cp = cc
                            if cp < Cfull:
                                MstA[32 * j + cc, m] = powers[Cfull - 1 - cp][i, j]
                    if colh == 1:
                        for j in range(4):
                            for cc in range(32):
                                cp = 32 + cc
                                if cp < Cfull:
                                    MstB[32 * j + cc, m - 128] = powers[Cfull - 1 - cp][i, j]

        ma64 = ma.astype(np.float64)
        hm = np.zeros(L + 1, np.float64)
        hm[:L] += ma64[1] * h[:L]
        hm[1:] += ma64[0] * h[:L]
        idx = np.arange(L)
        lagm = idx[None, :] - idx[:, None]  # tau - k
        Hm = np.where(lagm >= 0, hm[np.clip(lagm, 0, L)], 0.0)
        Hx = np.where(lagm >= 0, float(xcoef[0]) * h[np.clip(lagm, 0, L - 1)], 0.0)
        hb = (ma64[0] * h[:L]).reshape(1, L)           # weight of y[cL-1] at tau
        rd = (float(drift[0]) * np.cumsum(h[:L])).reshape(L, 1)

        # boundary-term state feed: d_c excludes the boundary contribution
        # (it is injected at phase 2), so states get it via W @ U where
        # U[c', b] = y[b, 128 c' + 127] and w_{C,c'} = Psi^{C-1-c'} @ hbv.
        hbv = hb[0, L - 4:]                            # [4]
        wvecs = {}
        for p in range(C):
            wvecs[p] = powers[p] @ hbv                 # [4] (i-indexed)
        WA = np.zeros((HALF, 256), np.float64)         # rows c' in half0
        WB = np.zeros((HALF, 128), np.float64)         # rows c' in half1
        # boundary of chunk c' uses u_{c'-1}; only c' >= 1 has one, so the
        # weight of u_p in s_C is Psi^{C-2-p} @ hbv for p <= C-2.
        for colh in range(2):
            for i in range(4):
                for CCn in range(HALF):
                    Cfull = HALF * colh + CCn
                    col = 128 * colh + HALF * i + CCn
                    for cc in range(HALF):
                        if cc <= Cfull - 2:            # u_p with p = cc (half0)
                            WA[cc, col] = wvecs[Cfull - 2 - cc][i]
                        if colh == 1 and HALF + cc <= Cfull - 2:
                            WB[cc, col - 128] = wvecs[Cfull - 2 - (HALF + cc)][i]

        rho = float(np.max(np.abs(np.roots([1.0, -c[3], -c[2], -c[1], -c[0]]))))
        Gaux = Gm / max(rho, 1.0) ** AUX_EXPO

        # main inject lhsT [5, 128]: rows 0-3 g_j (state response), row 4 hb
        # (boundary y[cL-1] response). The +/- aux pair accumulates into the
        # same PSUM as two separate f32r matmuls: exact cancellation when
        # finite, inf + (-inf) = NaN in PSUM on overflow (PE in-array K-chain
        # does NOT form NaN, so the pair must be PSUM-level).
        Gmain = np.zeros((128, L), np.float64)
        Gmain[0:4, :] = Gm.T
        Gmain[4, :] = hb[0]
        GauxP = np.zeros((128, L), np.float64)
        GauxP[0:4, :] = Gaux.T
        GauxN = -GauxP

        f = lambda x: np.ascontiguousarray(np.asarray(x, np.float32))
        return dict(
            Hm=f(Hm), Hx=f(Hx), rd=f(rd), Gmain=f(Gmain),
            GauxP=f(GauxP), GauxN=f(GauxN),
            MstA=f(MstA), MstB=f(MstB), WA=f(WA), WB=f(WB),
        )


def _r(ap, want):
    """optionally bitcast an AP to float32r"""
    return ap.bitcast(F32R) if want else ap


def _build_program():
    nc = bacc.Bacc("TRN2", target_bir_lowering=False, debug=False,
                   num_devices=NCORES)

    # y/ex arrive chunk-transposed from the host: row k, column 128*c + b
    # holds y[b, 128*c + k] (so conv matmuls read them directly as rhs)
    y_d = nc.dram_tensor("y", [BC, T_LEN], F32, kind="ExternalInput")
    ex_d = nc.dram_tensor("ex", [BC, T_LEN], F32, kind="ExternalInput")
    Hm_d = nc.dram_tensor("Hm", [L, L], F32, kind="ExternalInput")
    Hx_d = nc.dram_tensor("Hx", [L, L], F32, kind="ExternalInput")
    rd_d = nc.dram_tensor("rd", [L, 1], F32, kind="ExternalInput")
    Gm_d = nc.dram_tensor("Gmain", [128, L], F32, kind="ExternalInput")
    GxP_d = nc.dram_tensor("GauxP", [128, L], F32R, kind="ExternalInput")
    GxN_d = nc.dram_tensor("GauxN", [128, L], F32R, kind="ExternalInput")
    MstA_d = nc.dram_tensor("MstA", [128, 256], F32, kind="ExternalInput")
    MstB_d = nc.dram_tensor("MstB", [128, 128], F32, kind="ExternalInput")
    WA_d = nc.dram_tensor("WA", [HALF, 256], F32, kind="ExternalInput")
    WB_d = nc.dram_tensor("WB", [HALF, 128], F32, kind="ExternalInput")
    zed_d = nc.dram_tensor("zed", [1, 128], F32, kind="ExternalInput")
    out_d = nc.dram_tensor("out_t", [T_LEN, BC], F32, kind="ExternalOutput")

    HB = HALF * 128  # 4096 columns per half

    with tile.TileContext(nc) as tc:
        with (
            tc.tile_pool(name="consts", bufs=1) as consts,
            tc.tile_pool(name="persist", bufs=1) as persist,

            tc.tile_pool(name="outring", bufs=3) as outring,
            tc.tile_pool(name="small", bufs=1) as small,
            tc.tile_pool(name="pcv", bufs=3, space="PSUM") as pcv,
            tc.tile_pool(name="pin", bufs=3, space="PSUM") as pin,
            tc.tile_pool(name="psp", bufs=1, space="PSUM") as psp,
            tc.tile_pool(name="dram", bufs=1, space="DRAM") as dpool,
        ):
            Hm_t = consts.tile([L, L], F32, tag="Hm")
            Hx_t = consts.tile([L, L], F32, tag="Hx")
            rd_t = consts.tile([L, 1], F32, tag="rd")
            Gm_t = consts.tile([128, L], F32, tag="Gm")
            GxP_t = consts.tile([128, L], F32R, tag="GxP")
            GxN_t = consts.tile([128, L], F32R, tag="GxN")
            MstA_t = consts.tile([128, 256], F32, tag="MstA")
            MstB_t = consts.tile([128, 128], F32, tag="MstB")
            WA_t = consts.tile([HALF, 256], F32, tag="WA")
            WB_t = consts.tile([HALF, 128], F32, tag="WB")
            def load_early_consts():
                for t_, d_ in ((Hm_t, Hm_d), (Hx_t, Hx_d), (rd_t, rd_d)):
                    nc.sync.dma_start(t_[:], d_[:])

            def load_late_consts():
                # the half-0 state matmuls need these two first; the scalar
                # ring delivers them without queueing behind the bulk inputs
                nc.scalar.dma_start(MstA_t[:], MstA_d[:])
                nc.scalar.dma_start(WA_t[:], WA_d[:])
                for t_, d_ in ((Gm_t, Gm_d), (GxP_t, GxP_d), (GxN_t, GxN_d),
                               (MstB_t, MstB_d), (WB_t, WB_d)):
                    nc.scalar.dma_start(t_[:], d_[:])

            yT = persist.tile([128, T_LEN], F32, tag="yT")
            exT = persist.tile([128, T_LEN], F32, tag="exT")
            pT = persist.tile([128, T_LEN], F32, tag="pT")

            D0 = small.tile([128, 128], F32, tag="D0")
            D1 = small.tile([128, 128], F32, tag="D1")
            Sp0 = small.tile([128, 128], F32, tag="Sp0")
            Sp1 = small.tile([128, 128], F32, tag="Sp1")
            U0 = small.tile([HALF, 128], F32, tag="U0")
            U1 = small.tile([HALF, 128], F32, tag="U1")
            # inject rhs: S2m rows 0-3 states + row 4 boundary-u (f32);
            # S2x rows 0-3 states again, typed f32r for the aux pair
            # single shared inject rhs, typed f32r (the f32r aux matmuls
            # need f32r-typed producers; the fp32 main matmul reads a bitcast)
            S2m = small.tile([128, T_LEN], F32R, tag="S2m")
            # zero the padding rows once (K padded to 128 so inject matmuls
            # stream at full rate; zero weights x zero rhs contribute nothing)
            nc.gpsimd.memset(S2m[:].bitcast(F32), 0.0)

            scrD_0 = dpool.tile([1, 4 * HB], F32, tag="scrD0")
            scrD_1 = dpool.tile([1, 4 * HB], F32, tag="scrD1")
            scrS_0 = dpool.tile([1, 4 * HB], F32, tag="scrS0")
            scrS_1 = dpool.tile([1, 4 * HB], F32, tag="scrS1")
            scrD = [scrD_0, scrD_1]
            scrS = [scrS_0, scrS_1]

            def load_inputs():
                # ramped slice sizes: fast start, then efficient bulk
                bounds = [0, 512, 1536, 3072, 4608, 6400, 8192]
                for a, b in zip(bounds[:-1], bounds[1:]):
                    nc.sync.dma_start(yT[:, a:b], y_d[:, a:b])
                    nc.sync.dma_start(exT[:, a:b], ex_d[:, a:b])

            def phase1(g):
                sl = slice(g * 512, (g + 1) * 512)
                pp = pcv.tile([128, 512], F32, tag="pcv")
                nc.tensor.matmul(pp[:], Hm_t[:], yT[:, sl],
                                 start=True, stop=False)
                nc.tensor.matmul(pp[:], Hx_t[:], exT[:, sl],
                                 start=False, stop=True)
                if g % 2 == 0:
                    nc.vector.tensor_scalar_add(pT[:, sl], pp[:], rd_t[:, 0:1])
                else:
                    nc.scalar.activation(pT[:, sl], pp[:],
                                         mybir.ActivationFunctionType.Identity,
                                         bias=rd_t[:, 0:1], scale=1.0)

            def load_late_consts():
                # the half-0 state matmuls need these two first; the scalar
                # ring delivers them without queueing behind the bulk inputs
                nc.scalar.dma_start(MstA_t[:], MstA_d[:])
                nc.scalar.dma_start(WA_t[:], WA_d[:])
                for t_, d_ in ((Gm_t, Gm_d), (GxP_t, GxP_d), (GxN_t, GxN_d),
                               (MstB_t, MstB_d), (WB_t, WB_d)):
                    nc.scalar.dma_start(t_[:], d_[:])

            yT = persist.tile([128, T_LEN], F32, tag="yT")
            exT = persist.tile([128, T_LEN], F32, tag="exT")
            pT = persist.tile([128, T_LEN], F32, tag="pT")

            D0 = small.tile([128, 128], F32, tag="D0")
            D1 = small.tile([128, 128], F32, tag="D1")
            Sp0 = small.tile([128, 128], F32, tag="Sp0")
            Sp1 = small.tile([128, 128], F32, tag="Sp1")
            U0 = small.tile([HALF, 128], F32, tag="U0")
            U1 = small.tile([HALF, 128], F32, tag="U1")
            # inject rhs: S2m rows 0-3 states + row 4 boundary-u (f32);
            # S2x rows 0-3 states again, typed f32r for the aux pair
            # single shared inject rhs, typed f32r (the f32r aux matmuls
            # need f32r-typed producers; the fp32 main matmul reads a bitcast)
            S2m = small.tile([128, T_LEN], F32R, tag="S2m")
            # zero the padding rows once (K padded to 128 so inject matmuls
            # stream at full rate; zero weights x zero rhs contribute nothing)
            nc.gpsimd.memset(S2m[:].bitcast(F32), 0.0)

            scrD_0 = dpool.tile([1, 4 * HB], F32, tag="scrD0")
            scrD_1 = dpool.tile([1, 4 * HB], F32, tag="scrD1")
            scrS_0 = dpool.tile([1, 4 * HB], F32, tag="scrS0")
            scrS_1 = dpool.tile([1, 4 * HB], F32, tag="scrS1")
            scrD = [scrD_0, scrD_1]
            scrS = [scrS_0, scrS_1]

            def load_inputs():
                # ramped slice sizes: fast start, then efficient bulk
                bounds = [0, 512, 1536, 3072, 4608, 6400, 8192]
                for a, b in zip(bounds[:-1], bounds[1:]):
                    nc.sync.dma_start(yT[:, a:b], y_d[:, a:b])
                    nc.sync.dma_start(exT[:, a:b], ex_d[:, a:b])

            def phase1(g):
                sl = slice(g * 512, (g + 1) * 512)
                pp = pcv.tile([128, 512], F32, tag="pcv")
                nc.tensor.matmul(pp[:], Hm_t[:], yT[:, sl],
                                 start=True, stop=False)
                nc.tensor.matmul(pp[:], Hx_t[:], exT[:, sl],
                                 start=False, stop=True)
                if g % 2 == 0:
                    nc.vector.tensor_scalar_add(pT[:, sl], pp[:], rd_t[:, 0:1])
                else:
                    nc.scalar.activation(pT[:, sl], pp[:],
                                         mybir.ActivationFunctionType.Identity,
                                         bias=rd_t[:, 0:1], scale=1.0)

            def preload_state_rhs():
                # U rows and the boundary rhs row come straight from input DRAM
                nc.sync.dma_start(
                    U0[:], y_d[127:128, 0:HB]
                    .rearrange("one (p b) -> (one p) b", b=128))
                nc.sync.dma_start(
                    U1[:], y_d[127:128, HB:2 * HB]
                    .rearrange("one (p b) -> (one p) b", b=128))
                nc.sync.dma_start(S2m[4:5, 0:128], zed_d[:].bitcast(F32R))
                nc.sync.dma_start(S2m[4:5, 128:2 * HB],
                                  y_d[127:128, 0:2 * HB - 128].bitcast(F32R))

            def hs_gather(h):
                dma_eng = (nc.gpsimd.dma_start if h == 0
                           else nc.scalar.dma_start)
                cols = slice(h * HB, (h + 1) * HB)
                # single batched d-gather: [4 part, 4096] -> flat j-major
                dma_eng(
                    scrD[h][:].rearrange("one (j n) -> (one j) n", n=HB),
                    pT[124:128, cols])
                D = D0 if h == 0 else D1
                dma_eng(
                    D[:], scrD[h][:].rearrange("one (p b) -> (one p) b", b=128))

            def hs_compute(h):
                dma_eng = (nc.gpsimd.dma_start if h == 0
                           else nc.scalar.dma_start)
                cols = slice(h * HB, (h + 1) * HB)
                ps = psp.tile([128, 128], F32, tag="psp", name=f"ps{h}")
                if h == 0:
                    nc.tensor.matmul(ps[:], MstA_t[:, 0:128], D0[:],
                                     start=True, stop=False)
                    nc.tensor.matmul(ps[:], WA_t[:, 0:128], U0[:],
                                     start=False, stop=True)
                else:
                    nc.tensor.matmul(ps[:], MstA_t[:, 128:256], D0[:],
                                     start=True, stop=False)
                    nc.tensor.matmul(ps[:], MstB_t[:], D1[:],
                                     start=False, stop=False)
                    nc.tensor.matmul(ps[:], WA_t[:, 128:256], U0[:],
                                     start=False, stop=False)
                    nc.tensor.matmul(ps[:], WB_t[:], U1[:],
                                     start=False, stop=True)
                Sp = Sp0 if h == 0 else Sp1
                nc.vector.tensor_copy(Sp[:], ps[:])
                # single batched shuffle: Sp [128,128] -> flat i-major -> rows
                dma_eng(
                    scrS[h][:].rearrange("one (i n) -> (one i) n", n=HB),
                    Sp[:])
                dma_eng(
                    S2m[0:4, cols],
                    scrS[h][:].rearrange("one (i n) -> (one i) n",
                                         n=HB).bitcast(F32R))

            og_hold = [None]

            def phase2(g):
                sl = slice(g * 512, (g + 1) * 512)
                pj = pin.tile([128, 512], F32, tag="pin")
                nc.tensor.matmul(pj[:], Gm_t[:], S2m[:, sl].bitcast(F32),
                                 start=True, stop=False)
                nc.tensor.matmul(pj[:], GxP_t[:], S2m[:, sl],
                                 start=False, stop=False)
                nc.tensor.matmul(pj[:], GxN_t[:], S2m[:, sl],
                                 start=False, stop=True)
                if g % 2 == 0:
                    og_hold[0] = outring.tile([128, 1024], F32, tag="og",
                                              name=f"og{g}")
                og = og_hold[0]
                half = (g % 2) * 512
                nc.vector.tensor_add(og[:, half:half + 512], pT[:, sl], pj[:])
                if g >= G - 2:
                    # final groups: ship each half as soon as its add lands
                    nc.sync.dma_start(
                        out_d[sl, :].rearrange("(c t) b -> t c b", c=NG),
                        og[:, half:half + 512]
                        .rearrange("t (c b) -> t c b", c=NG))
                elif g % 2 == 1:
                    osl = slice((g - 1) * 512, (g + 1) * 512)
                    nc.sync.dma_start(
                        out_d[osl, :].rearrange("(c t) b -> t c b", c=2 * NG),
                        og[:].rearrange("t (c b) -> t c b", c=2 * NG))

            junk = small.tile([1, 4], F32, tag="junk")

            def warmup_dummies(n, tag):
                pdum = psp.tile([128, 512], F32, tag="pdum", name=f"pd{tag}")
                for i in range(n):
                    nc.tensor.matmul(pdum[:], Hm_t[:], yT[:, 0:512],
                                     start=(i == 0), stop=(i == n - 1))
                nc.vector.tensor_copy(junk[:], pdum[0:1, 0:4])
                nc.gpsimd.dma_start(scrD[0][0:1, 0:4], junk[:])

            load_early_consts()
            load_inputs()
            preload_state_rhs()
            load_late_consts()
            for g in range(G):
                phase1(g)
                if g == 7:
                    hs_gather(0)
                if g == 12:
                    hs_compute(0)
            hs_gather(1)
            for g in range(G // 2):
                phase2(g)
                if g == 3:
                    hs_compute(1)
            for g in range(G // 2, G):
                phase2(g)

    nc.finalize()
    return nc


_CACHE = {}
TRACE = False        # set True (e.g. from test.py) to capture neuron-profile
LAST_RESULT = None   # BassKernelResults of the last run


def kernel(y, exog, ar, ma, xcoef, drift):
    y = np.asarray(y)
    exog = np.asarray(exog)
    pc = _host_precompute(np.asarray(ar), np.asarray(ma),
                          np.asarray(xcoef), np.asarray(drift))

    if "nc" not in _CACHE:
        _CACHE["nc"] = _build_program()
    nc = _CACHE["nc"]

    # chunk-transpose on host: per core [k, 128*c + b] = y[b, 128*c + k]
    y4 = (y.reshape(NCORES, BC, C, L).transpose(0, 3, 2, 1)
          .reshape(NCORES, L, T_LEN).astype(np.float32))
    ex4 = (exog.reshape(NCORES, BC, C, L).transpose(0, 3, 2, 1)
           .reshape(NCORES, L, T_LEN).astype(np.float32))

    in_maps = []
    for i in range(NCORES):
        m = {
            "y": np.ascontiguousarray(y4[i]),
            "ex": np.ascontiguousarray(ex4[i]),
            "zed": np.zeros((1, 128), np.float32),
        }
        m.update(pc)  # const names match dram tensor names
        in_maps.append(m)

    res = run_bass_kernel_spmd(nc, in_maps, core_ids=list(range(NCORES)),
                               trace=TRACE)
    global LAST_RESULT
    LAST_RESULT = res

    out = np.empty((B_TOT, 1, T_LEN), np.float32)
    for i in range(NCORES):
        out[i * BC:(i + 1) * BC, 0, :] = res.results[i]["out_t"].T
    return out


if __name__ == "__main__":
    d = np.load("/root/problem/ref_inp.npz")
    inp = {k: d[k] for k in d.files}
    o = kernel(**inp)
    print("kernel ran, out shape", o.shape)
    ref = np.load("/root/problem/ref_out_np.npy")
    r = ref[:, 0, :]
    oo = o[:, 0, :]
    bf = np.isfinite(r) & np.isfinite(oo)
    rel = np.abs(oo - r)[bf] / (np.abs(r)[bf] + 1e-30)
    print("finite rel: max %.3g mean %.3g" % (rel.max(), rel.mean()))
    print("class mism:", (np.isfinite(r) != np.isfinite(oo)).sum(),
          "nan mism:", (np.isnan(r) != np.isnan(oo)).sum())
    np.save("/root/problem/last_out.npy", o)
    nm = (np.isnan(r) != np.isnan(oo))
    ts = np.where(nm.any(axis=0))[0]
    if len(ts):
        print("nan-mismatch t range", ts.min(), ts.max(), "ncols", len(ts))
    # finite-region error per chunk (diagnose which chunks are wrong)
    for c in [0, 1, 2, 3, 10, 32, 63]:
        s = slice(c * 128, (c + 1) * 128)
        b = np.isfinite(r[:, s]) & np.isfinite(oo[:, s])
        if b.any():
            e = np.abs(oo[:, s] - r[:, s])[b] / (np.abs(r[:, s])[b] + 1e-30)
            print(f"chunk {c}: n_finite {b.sum()} relmax {e.max():.3g} "
                  f"relmean {e.mean():.3g}")
